# revision 4
# baseline (speedup 1.0000x reference)
"""MultiHead scaled-dot-product attention on 8 Trainium2 NeuronCores.

Sharding: 8 cores = 4 batches x 2 head-halves. Core c handles batch c//2 and
heads [8*(c%2), 8*(c%2)+8) (512 of the 1024 feature columns of WQ/WK/WV,
and 512 rows of WO.T). Each core emits a partial output projection
(z_local @ wo.T_local, no bias); the host sums the two partials per batch and
adds bo_eff = bo + bv @ wo.T (the V-bias folds out of attention because
softmax rows sum to 1).

On-device layout per core (all matmul operands bf16, fp32 PSUM accumulate):
  Q^T, K^T: [512 feat, 2048 seq] (features on partitions, 4 head-pair tiles)
  V:        [2048 seq, 8 heads, 64+1] with a ones column -> PV matmul
            computes both z^T and the softmax denominator in one pass.
  S^T = K^T.T-chunks @ Q^T  (keys on partitions, queries free) so softmax
        normalization needs no P transpose; exp runs on ScalarE reading PSUM
        with the 1/sqrt(64) scale fused; mask applied as a 0/1 bf16 multiply.
  z^T feeds the WO matmul as the stationary operand directly.
"""

import numpy as np
import ml_dtypes

import concourse.bass as bass
import concourse.mybir as mybir
from concourse import bacc
from concourse.bass_utils import run_bass_kernel_spmd
from concourse.tile import TileContext

BF16 = mybir.dt.bfloat16
F32 = mybir.dt.float32
NPBF16 = ml_dtypes.bfloat16

DIM = 1024
HEADS = 16
HD = 64
BS = 4
SQ = 2048
SK = 2048
LOCF = 512     # features per core (8 heads x 64)
NCH = DIM // 128   # 8 contraction chunks for projections
NQT = SQ // 512    # 4 query tiles
NKT = SK // 128    # 16 key tiles
NHP = 4            # head pairs per core

_BUILT = None


def _build():
    nc = bacc.Bacc("TRN2", target_bir_lowering=False, debug=False, num_devices=8)

    xqT = nc.dram_tensor("xqT", [DIM, SQ], BF16, kind="ExternalInput").ap()
    xkT = nc.dram_tensor("xkT", [DIM, SK], BF16, kind="ExternalInput").ap()
    xvT = nc.dram_tensor("xvT", [DIM, SK], BF16, kind="ExternalInput").ap()
    maskT = nc.dram_tensor("maskT", [SK, SQ], BF16, kind="ExternalInput").ap()
    wqT = nc.dram_tensor("wqT", [DIM, LOCF], BF16, kind="ExternalInput").ap()
    wkT = nc.dram_tensor("wkT", [DIM, LOCF], BF16, kind="ExternalInput").ap()
    wvT = nc.dram_tensor("wvT", [DIM, LOCF], BF16, kind="ExternalInput").ap()
    woT = nc.dram_tensor("woT", [LOCF, DIM], BF16, kind="ExternalInput").ap()
    bq_d = nc.dram_tensor("bq", [LOCF, 1], F32, kind="ExternalInput").ap()
    bk_d = nc.dram_tensor("bk", [LOCF, 1], F32, kind="ExternalInput").ap()
    out_d = nc.dram_tensor("out", [SQ, DIM], F32, kind="ExternalOutput").ap()

    Exp = mybir.ActivationFunctionType.Exp

    with TileContext(nc) as tc:
        from contextlib import ExitStack
        with ExitStack() as ctx:
            singles = ctx.enter_context(tc.tile_pool(name="singles", bufs=1))
            xc_pool = ctx.enter_context(tc.tile_pool(name="xc", bufs=8))
            mask_pool = ctx.enter_context(tc.tile_pool(name="mask", bufs=2))
            pt_pool = ctx.enter_context(tc.tile_pool(name="pt", bufs=4))
            rb_pool = ctx.enter_context(tc.tile_pool(name="rb", bufs=2))
            r1_pool = ctx.enter_context(tc.tile_pool(name="r1", bufs=2))
            osb_pool = ctx.enter_context(tc.tile_pool(name="osb", bufs=3))
            ps2 = ctx.enter_context(tc.tile_pool(name="ps2", bufs=2, space="PSUM"))
            psz = ctx.enter_context(tc.tile_pool(name="psz", bufs=2, space="PSUM"))
            pso = ctx.enter_context(tc.tile_pool(name="pso", bufs=2, space="PSUM"))

            # --- weights / biases ---
            w_sb = {}
            for nm, dram in (("wq", wqT), ("wk", wkT), ("wv", wvT)):
                t = singles.tile([128, NCH, LOCF], BF16, tag=f"w_{nm}")
                nc.sync.dma_start(out=t, in_=dram.rearrange("(c p) f -> p c f", p=128))
                w_sb[nm] = t
            woT_sb = singles.tile([128, 4, DIM], BF16, tag="wo")
            nc.sync.dma_start(out=woT_sb, in_=woT.rearrange("(c p) f -> p c f", p=128))
            bq_sb = singles.tile([128, 4, 1], F32, tag="bq")
            nc.sync.dma_start(out=bq_sb, in_=bq_d.rearrange("(m p) o -> p m o", p=128))
            bk_sb = singles.tile([128, 4, 1], F32, tag="bk")
            nc.sync.dma_start(out=bk_sb, in_=bk_d.rearrange("(m p) o -> p m o", p=128))

            QT_sb = singles.tile([128, NHP, SQ], BF16, tag="QT")
            KT_sb = singles.tile([128, NHP, SK], BF16, tag="KT")
            ZT_sb = singles.tile([128, NHP, SQ], BF16, tag="ZT")
            V_sb = singles.tile([128, NKT, 8, HD + 1], BF16, tag="V")
            nc.vector.memset(V_sb[:, :, :, HD:HD + 1], 1.0)

            # --- Q^T / K^T projections: out[feat_tile, seq] ---
            for src, wkey, bias_sb, dst in (
                (xqT, "wq", bq_sb, QT_sb),
                (xkT, "wk", bk_sb, KT_sb),
            ):
                xcs = []
                for c in range(NCH):
                    xt = xc_pool.tile([128, SQ], BF16, tag="xc")
                    nc.sync.dma_start(out=xt, in_=src[c * 128:(c + 1) * 128, :])
                    xcs.append(xt)
                for m in range(NHP):
                    for n in range(NQT):
                        ps = pso.tile([128, 512], F32, tag="pso")
                        for c in range(NCH):
                            nc.tensor.matmul(
                                ps,
                                lhsT=w_sb[wkey][:, c, m * 128:(m + 1) * 128],
                                rhs=xcs[c][:, n * 512:(n + 1) * 512],
                                start=(c == 0),
                                stop=(c == NCH - 1),
                            )
                        nc.vector.tensor_scalar_add(
                            out=dst[:, m, n * 512:(n + 1) * 512],
                            in0=ps,
                            scalar1=bias_sb[:, m, :],
                        )

            # --- V projection: natural [seq, feat] layout (no bias: folded) ---
            xvs = []
            for c in range(NCH):
                xt = xc_pool.tile([128, SK], BF16, tag="xc")
                nc.sync.dma_start(out=xt, in_=xvT[c * 128:(c + 1) * 128, :])
                xvs.append(xt)
            for m in range(NKT):
                ps = pso.tile([128, 512], F32, tag="pso")
                for c in range(NCH):
                    nc.tensor.matmul(
                        ps,
                        lhsT=xvs[c][:, m * 128:(m + 1) * 128],
                        rhs=w_sb["wv"][:, c, :],
                        start=(c == 0),
                        stop=(c == NCH - 1),
                    )
                nc.vector.tensor_copy(
                    out=V_sb[:, m, :, 0:HD],
                    in_=ps.rearrange("p (h d) -> p h d", h=8),
                )

            # --- attention + output projection, per query tile ---
            for qt in range(NQT):
                q_sl = slice(qt * 512, (qt + 1) * 512)
                mk = mask_pool.tile([128, NKT, 512], BF16, tag="mask")
                nc.sync.dma_start(
                    out=mk,
                    in_=maskT.rearrange("(kt p) q -> p kt q", p=128)[:, :, q_sl],
                )
                for hp in range(NHP):
                    pz = [
                        psz.tile([HD + 1, 512], F32, tag="psz", name="pz0"),
                        psz.tile([HD + 1, 512], F32, tag="psz", name="pz1"),
                    ]
                    for g in range(NKT // 2):
                        pss = [
                            ps2.tile([128, 2, 512], F32, tag="ps2", name="pssA"),
                            ps2.tile([128, 2, 512], F32, tag="ps2", name="pssB"),
                        ]
                        for j in range(2):
                            kt = 2 * g + j
                            for hl in range(2):
                                p_sl = slice(hl * 64, (hl + 1) * 64)
                                nc.tensor.matmul(
                                    pss[hl][:, j, :],
                                    lhsT=KT_sb[p_sl, hp, kt * 128:(kt + 1) * 128],
                                    rhs=QT_sb[p_sl, hp, q_sl],
                                    start=True,
                                    stop=True,
                                )
                        for hl in range(2):
                            pt = pt_pool.tile([128, 2, 512], BF16, tag="pt")
                            nc.scalar.activation(
                                out=pt, in_=pss[hl], func=Exp, scale=0.125
                            )
                            nc.vector.tensor_mul(
                                out=pt, in0=pt, in1=mk[:, 2 * g:2 * g + 2, :]
                            )
                            for j in range(2):
                                kt = 2 * g + j
                                nc.tensor.matmul(
                                    pz[hl],
                                    lhsT=V_sb[:, kt, hp * 2 + hl, :],
                                    rhs=pt[:, j, :],
                                    start=(kt == 0),
                                    stop=(kt == NKT - 1),
                                )
                    for hl in range(2):
                        p_sl = slice(hl * 64, (hl + 1) * 64)
                        r1 = r1_pool.tile([1, 512], F32, tag="r1")
                        nc.vector.reciprocal(out=r1, in_=pz[hl][HD:HD + 1, :])
                        rb = rb_pool.tile([HD, 512], F32, tag="rb")
                        nc.gpsimd.partition_broadcast(rb, r1)
                        nc.vector.tensor_mul(
                            out=ZT_sb[p_sl, hp, q_sl],
                            in0=pz[hl][0:HD, :],
                            in1=rb,
                        )
                # output projection for this query tile's 4 seq subtiles
                for st in range(4):
                    s0 = qt * 512 + st * 128
                    for n in range(2):
                        po = pso.tile([128, 512], F32, tag="pso")
                        for c4 in range(NHP):
                            nc.tensor.matmul(
                                po,
                                lhsT=ZT_sb[:, c4, s0:s0 + 128],
                                rhs=woT_sb[:, c4, n * 512:(n + 1) * 512],
                                start=(c4 == 0),
                                stop=(c4 == NHP - 1),
                            )
                        ot = osb_pool.tile([128, 512], F32, tag="osb")
                        nc.vector.tensor_copy(out=ot, in_=po)
                        nc.sync.dma_start(
                            out=out_d[s0:s0 + 128, n * 512:(n + 1) * 512], in_=ot
                        )

    nc.compile()
    return nc


def _get_nc():
    global _BUILT
    if _BUILT is None:
        _BUILT = _build()
    return _BUILT


def _prep_in_maps(x_q, x_k, x_v, mask, wq, bq, wk, bk, wv, wo):
    per_batch = []
    for b in range(BS):
        per_batch.append({
            "xqT": np.ascontiguousarray(x_q[b].T).astype(NPBF16),
            "xkT": np.ascontiguousarray(x_k[b].T).astype(NPBF16),
            "xvT": np.ascontiguousarray(x_v[b].T).astype(NPBF16),
            "maskT": np.ascontiguousarray((mask[b] != 0).T).astype(NPBF16),
        })
    per_half = []
    for hh in range(2):
        sl = slice(hh * LOCF, (hh + 1) * LOCF)
        per_half.append({
            "wqT": np.ascontiguousarray(wq.T[:, sl]).astype(NPBF16),
            "wkT": np.ascontiguousarray(wk.T[:, sl]).astype(NPBF16),
            "wvT": np.ascontiguousarray(wv.T[:, sl]).astype(NPBF16),
            "woT": np.ascontiguousarray(wo.T[sl, :]).astype(NPBF16),
            "bq": np.ascontiguousarray(bq[sl]).reshape(LOCF, 1).astype(np.float32),
            "bk": np.ascontiguousarray(bk[sl]).reshape(LOCF, 1).astype(np.float32),
        })
    in_maps = []
    for c in range(8):
        m = dict(per_batch[c // 2])
        m.update(per_half[c % 2])
        in_maps.append(m)
    return in_maps


def _run(inputs, trace=False):
    x_q = np.asarray(inputs["x_q"], dtype=np.float32)
    x_k = np.asarray(inputs["x_k"], dtype=np.float32)
    x_v = np.asarray(inputs["x_v"], dtype=np.float32)
    mask = np.asarray(inputs["mask"])
    wq = np.asarray(inputs["wq"], dtype=np.float32)
    bq = np.asarray(inputs["bq"], dtype=np.float32)
    wk = np.asarray(inputs["wk"], dtype=np.float32)
    bk = np.asarray(inputs["bk"], dtype=np.float32)
    wv = np.asarray(inputs["wv"], dtype=np.float32)
    bv = np.asarray(inputs["bv"], dtype=np.float32)
    wo = np.asarray(inputs["wo"], dtype=np.float32)
    bo = np.asarray(inputs["bo"], dtype=np.float32)

    nc = _get_nc()
    in_maps = _prep_in_maps(x_q, x_k, x_v, mask, wq, bq, wk, bk, wv, wo)
    kw = {}
    if trace:
        kw = dict(trace=True, trace_cores=[0], stitch_traces=False)
    res = run_bass_kernel_spmd(nc, in_maps, core_ids=list(range(8)), **kw)

    bo_eff = (bo + bv @ wo.T).astype(np.float32)
    out = np.empty((BS, SQ, DIM), np.float32)
    for b in range(BS):
        out[b] = res.results[2 * b]["out"] + res.results[2 * b + 1]["out"] + bo_eff
    return out, res


def kernel(**inputs):
    return _run(inputs, trace=False)[0]


# revision 15
# speedup vs baseline: 1.1189x; 1.1189x over previous
"""MultiHead scaled-dot-product attention on 8 Trainium2 NeuronCores.

Sharding: 8 cores = 4 batches x 2 head-halves. Core c handles batch c//2 and
heads [8*(c%2), 8*(c%2)+8) (512 of the 1024 feature columns of WQ/WK/WV,
and 512 rows of WO.T). Each core emits a partial output projection
(z_local @ wo.T_local, no bias); the host sums the two partials per batch and
adds bo_eff = bo + bv @ wo.T (the V-bias folds out of attention because
softmax rows sum to 1).

On-device layout per core (all matmul operands bf16, fp32 PSUM accumulate):
  Q^T, K^T: [512 feat, 2048 seq] (features on partitions, 4 head-pair tiles)
  V:        [2048 seq, 8 heads, 64+1] with a ones column -> PV matmul
            computes both z^T and the softmax denominator in one pass.
  S^T = K^T-chunks.T @ Q^T  (keys on partitions, queries free) so softmax
        needs no P transpose. Per key tile, the two heads of a pair run as
        row-packed (tile_position) matmuls into the two banks of one PSUM
        tile; one ScalarE exp covers both (scale=1/8 fused); the 0/1 bf16
        mask multiplies both via a step-0 broadcast AP.
  z^T feeds the WO matmul as the stationary operand directly. Softmax
  denominators are collected per query-tile and reciprocated in one batched
  DVE op to keep the slow reciprocal off the PE critical path (PE gaps
  > ~3.4us trip the HAM clock gate down to 1.2 GHz).
"""

import numpy as np
import ml_dtypes

import concourse.bass as bass
import concourse.mybir as mybir
from concourse import bacc
from concourse.bass_utils import run_bass_kernel_spmd
from concourse.tile import TileContext

BF16 = mybir.dt.bfloat16
F32 = mybir.dt.float32
NPBF16 = ml_dtypes.bfloat16

DIM = 1024
HEADS = 16
HD = 64
BS = 4
SQ = 2048
SK = 2048
LOCF = 512     # features per core (8 heads x 64)
NCH = DIM // 128   # 8 contraction chunks for projections
NQT = SQ // 512    # 4 query tiles
NKT = SK // 128    # 16 key tiles
NHP = 4            # head pairs per core

_BUILT = None


def _build():
    nc = bacc.Bacc("TRN2", target_bir_lowering=False, debug=False, num_devices=8)

    xqT = nc.dram_tensor("xqT", [DIM, SQ], BF16, kind="ExternalInput").ap()
    xkT = nc.dram_tensor("xkT", [DIM, SK], BF16, kind="ExternalInput").ap()
    xvT = nc.dram_tensor("xvT", [DIM, SK], BF16, kind="ExternalInput").ap()
    maskT = nc.dram_tensor("maskT", [SK, SQ], BF16, kind="ExternalInput").ap()
    wqT = nc.dram_tensor("wqT", [DIM, LOCF], BF16, kind="ExternalInput").ap()
    wkT = nc.dram_tensor("wkT", [DIM, LOCF], BF16, kind="ExternalInput").ap()
    wvT = nc.dram_tensor("wvT", [DIM, LOCF], BF16, kind="ExternalInput").ap()
    woT = nc.dram_tensor("woT", [LOCF, DIM], BF16, kind="ExternalInput").ap()
    bq_d = nc.dram_tensor("bq", [LOCF, 1], F32, kind="ExternalInput").ap()
    bk_d = nc.dram_tensor("bk", [LOCF, 1], F32, kind="ExternalInput").ap()
    out_d = nc.dram_tensor("out", [SQ, DIM], F32, kind="ExternalOutput").ap()

    Exp = mybir.ActivationFunctionType.Exp

    with TileContext(nc) as tc:
        from contextlib import ExitStack
        with ExitStack() as ctx:
            singles = ctx.enter_context(tc.tile_pool(name="singles", bufs=1))
            mask_pool = ctx.enter_context(tc.tile_pool(name="mask", bufs=3))
            dram_pool = ctx.enter_context(
                tc.tile_pool(name="drp", bufs=2, space="DRAM")
            )
            pt_pool = ctx.enter_context(tc.tile_pool(name="pt", bufs=6))
            dn_pool = ctx.enter_context(tc.tile_pool(name="dn", bufs=2))
            rb_pool = ctx.enter_context(tc.tile_pool(name="rb", bufs=3))
            osb_pool = ctx.enter_context(tc.tile_pool(name="osb", bufs=3))
            # PSUM: ps2 = 3 x 2-bank score tiles, psx = 2 x 1-bank tiles
            # shared by projections, PV accumulators, and WO outputs.
            ps2 = ctx.enter_context(tc.tile_pool(name="ps2", bufs=3, space="PSUM"))
            psx = ctx.enter_context(tc.tile_pool(name="psx", bufs=2, space="PSUM"))

            # --- weights / biases ---
            w_sb = {}
            for nm, dram in (("wq", wqT), ("wk", wkT), ("wv", wvT)):
                t = singles.tile([128, NCH, LOCF], BF16, tag=f"w_{nm}", name=f"w_{nm}")
                nc.sync.dma_start(out=t, in_=dram.rearrange("(c p) f -> p c f", p=128))
                w_sb[nm] = t
            woT_sb = singles.tile([128, 4, DIM], BF16, tag="wo")
            nc.sync.dma_start(out=woT_sb, in_=woT.rearrange("(c p) f -> p c f", p=128))
            bq_sb = singles.tile([128, 4, 1], F32, tag="bq")
            nc.sync.dma_start(out=bq_sb, in_=bq_d.rearrange("(m p) o -> p m o", p=128))
            bk_sb = singles.tile([128, 4, 1], F32, tag="bk")
            nc.sync.dma_start(out=bk_sb, in_=bk_d.rearrange("(m p) o -> p m o", p=128))

            QT_sb = singles.tile([128, NHP, SQ], BF16, tag="QT")
            KT_sb = singles.tile([128, NHP, SK], BF16, tag="KT")
            ZT_sb = singles.tile([128, NHP, SQ], BF16, tag="ZT")
            V_sb = singles.tile([128, NKT, 8, HD + 1], BF16, tag="V")
            nc.vector.memset(V_sb[:, :, :, HD:HD + 1], 1.0)

            # --- phase A: projections, streamed in seq halves (1024 cols) ---
            HS = SQ // 2
            with tc.tile_pool(name="xc", bufs=8) as xc_pool:
                # K^T projection first (attention needs all keys first)
                for half in range(2):
                    h_sl = slice(half * HS, (half + 1) * HS)
                    xks = []
                    for c in range(NCH):
                        xt = xc_pool.tile([128, HS], BF16, tag="xc", name="xk")
                        nc.sync.dma_start(out=xt, in_=xkT[c * 128:(c + 1) * 128, h_sl])
                        xks.append(xt)
                    for m in range(NHP):
                        for nl in range(2):
                            n = half * 2 + nl
                            ps = psx.tile([128, 512], F32, tag="psx", name="psk")
                            for c in range(NCH):
                                nc.tensor.matmul(
                                    ps,
                                    lhsT=w_sb["wk"][:, c, m * 128:(m + 1) * 128],
                                    rhs=xks[c][:, nl * 512:(nl + 1) * 512],
                                    start=(c == 0),
                                    stop=(c == NCH - 1),
                                )
                            nc.vector.tensor_scalar_add(
                                out=KT_sb[:, m, n * 512:(n + 1) * 512],
                                in0=ps,
                                scalar1=bk_sb[:, m, :],
                            )

                # V projection: natural [seq, feat] layout (bias folded out)
                for half in range(2):
                    h_sl = slice(half * HS, (half + 1) * HS)
                    xvs = []
                    for c in range(NCH):
                        xt = xc_pool.tile([128, HS], BF16, tag="xc", name="xv")
                        nc.sync.dma_start(out=xt, in_=xvT[c * 128:(c + 1) * 128, h_sl])
                        xvs.append(xt)
                    for ml in range(NKT // 2):
                        m = half * (NKT // 2) + ml
                        ps = psx.tile([128, 512], F32, tag="psx", name="psv")
                        for c in range(NCH):
                            nc.tensor.matmul(
                                ps,
                                lhsT=xvs[c][:, ml * 128:(ml + 1) * 128],
                                rhs=w_sb["wv"][:, c, :],
                                start=(c == 0),
                                stop=(c == NCH - 1),
                            )
                        nc.vector.tensor_copy(
                            out=V_sb[:, m, :, 0:HD],
                            in_=ps.rearrange("p (h d) -> p h d", h=8),
                        )

                # Q^T projection, seq-tile-major so qt=0 attention can start
                for half in range(2):
                    h_sl = slice(half * HS, (half + 1) * HS)
                    xqs = []
                    for c in range(NCH):
                        xt = xc_pool.tile([128, HS], BF16, tag="xc", name="xq")
                        nc.sync.dma_start(out=xt, in_=xqT[c * 128:(c + 1) * 128, h_sl])
                        xqs.append(xt)
                    for nl in range(2):
                        n = half * 2 + nl
                        for m in range(NHP):
                            ps = psx.tile([128, 512], F32, tag="psx", name="psq")
                            for c in range(NCH):
                                nc.tensor.matmul(
                                    ps,
                                    lhsT=w_sb["wq"][:, c, m * 128:(m + 1) * 128],
                                    rhs=xqs[c][:, nl * 512:(nl + 1) * 512],
                                    start=(c == 0),
                                    stop=(c == NCH - 1),
                                )
                            nc.vector.tensor_scalar_add(
                                out=QT_sb[:, m, n * 512:(n + 1) * 512],
                                in0=ps,
                                scalar1=bq_sb[:, m, :],
                            )

            # --- attention + output projection, per query tile ---
            for qt in range(NQT):
                q_sl = slice(qt * 512, (qt + 1) * 512)
                mks = []
                for kh in range(2):
                    mk = mask_pool.tile([128, NKT // 2, 512], BF16, tag="mask",
                                        name="mk")
                    nc.sync.dma_start(
                        out=mk,
                        in_=maskT.rearrange("(kt p) q -> p kt q", p=128)[
                            :, kh * (NKT // 2):(kh + 1) * (NKT // 2), q_sl],
                    )
                    mks.append(mk)
                dn = dn_pool.tile([8, 512], F32, tag="dn", name="dn")
                for hp in range(NHP):
                    pz = [
                        psx.tile([HD + 1, 512], F32, tag="psx", name="pz0"),
                        psx.tile([HD + 1, 512], F32, tag="psx", name="pz1"),
                    ]
                    for kt in range(NKT):
                        ss = ps2.tile([128, 2, 512], F32, tag="ps2", name="ss")
                        for hl in range(2):
                            p_sl = slice(hl * 64, (hl + 1) * 64)
                            nc.tensor.matmul(
                                ss[:, hl, :],
                                lhsT=KT_sb[p_sl, hp, kt * 128:(kt + 1) * 128],
                                rhs=QT_sb[p_sl, hp, q_sl],
                                start=True,
                                stop=True,
                            )
                        pt = pt_pool.tile([128, 2, 512], BF16, tag="pt", name="pt")
                        nc.scalar.activation(out=pt, in_=ss, func=Exp, scale=0.125)
                        mkh = mks[kt // (NKT // 2)]
                        ktl = kt % (NKT // 2)
                        nc.vector.tensor_mul(
                            out=pt,
                            in0=pt,
                            in1=mkh[:, ktl:ktl + 1, :].to_broadcast((128, 2, 512)),
                        )
                        for hl in range(2):
                            nc.tensor.matmul(
                                pz[hl],
                                lhsT=V_sb[:, kt, hp * 2 + hl, :],
                                rhs=pt[:, hl, :],
                                start=(kt == 0),
                                stop=(kt == NKT - 1),
                            )
                    for hl in range(2):
                        h = hp * 2 + hl
                        p_sl = slice(hl * 64, (hl + 1) * 64)
                        # stash unnormalized z^T; free the PSUM bank fast
                        nc.vector.tensor_copy(
                            out=ZT_sb[p_sl, hp, q_sl], in_=pz[hl][0:HD, :]
                        )
                        dnst = dn_pool.tile([1, 512], F32, tag="dnst", name="dnst")
                        nc.vector.tensor_copy(out=dnst, in_=pz[hl][HD:HD + 1, :])
                        nc.sync.dma_start(out=dn[h:h + 1, :], in_=dnst)
                # one batched reciprocal for all 8 heads of this q tile
                rall = dn_pool.tile([8, 512], F32, tag="rall", name="rall")
                nc.vector.reciprocal(out=rall, in_=dn)
                # bounce reciprocals through DRAM so the per-head broadcast
                # DMA can use a zero-stride partition dim (DRAM source only)
                dr = dram_pool.tile([8, 1, 512], F32, tag="dr", name="dr")
                nc.sync.dma_start(out=dr, in_=rall)
                for hp in range(NHP):
                    # rb partitions 0-63 = head hp*2 reciprocal, 64-127 = hp*2+1
                    rb = rb_pool.tile([128, 512], F32, tag="rb", name="rb")
                    nc.sync.dma_start(
                        out=rb,
                        in_=dr[2 * hp:2 * hp + 2, :, :].to_broadcast((2, HD, 512)),
                    )
                    nc.vector.tensor_mul(
                        out=ZT_sb[:, hp, q_sl],
                        in0=ZT_sb[:, hp, q_sl],
                        in1=rb,
                    )
                # output projection for this query tile
                for st in range(4):
                    s0 = qt * 512 + st * 128
                    for n in range(2):
                        po = psx.tile([128, 512], F32, tag="psx", name="po")
                        for c4 in range(NHP):
                            nc.tensor.matmul(
                                po,
                                lhsT=ZT_sb[:, c4, s0:s0 + 128],
                                rhs=woT_sb[:, c4, n * 512:(n + 1) * 512],
                                start=(c4 == 0),
                                stop=(c4 == NHP - 1),
                            )
                        ot = osb_pool.tile([128, 512], F32, tag="osb", name="ot")
                        nc.vector.tensor_copy(out=ot, in_=po)
                        nc.sync.dma_start(
                            out=out_d[s0:s0 + 128, n * 512:(n + 1) * 512], in_=ot
                        )

    nc.compile()
    return nc


def _get_nc():
    global _BUILT
    if _BUILT is None:
        _BUILT = _build()
    return _BUILT


def _prep_in_maps(x_q, x_k, x_v, mask, wq, bq, wk, bk, wv, wo):
    per_batch = []
    for b in range(BS):
        per_batch.append({
            "xqT": np.ascontiguousarray(x_q[b].T).astype(NPBF16),
            "xkT": np.ascontiguousarray(x_k[b].T).astype(NPBF16),
            "xvT": np.ascontiguousarray(x_v[b].T).astype(NPBF16),
            "maskT": np.ascontiguousarray((mask[b] != 0).T).astype(NPBF16),
        })
    per_half = []
    for hh in range(2):
        sl = slice(hh * LOCF, (hh + 1) * LOCF)
        per_half.append({
            "wqT": np.ascontiguousarray(wq.T[:, sl]).astype(NPBF16),
            "wkT": np.ascontiguousarray(wk.T[:, sl]).astype(NPBF16),
            "wvT": np.ascontiguousarray(wv.T[:, sl]).astype(NPBF16),
            "woT": np.ascontiguousarray(wo.T[sl, :]).astype(NPBF16),
            "bq": np.ascontiguousarray(bq[sl]).reshape(LOCF, 1).astype(np.float32),
            "bk": np.ascontiguousarray(bk[sl]).reshape(LOCF, 1).astype(np.float32),
        })
    in_maps = []
    for c in range(8):
        m = dict(per_batch[c // 2])
        m.update(per_half[c % 2])
        in_maps.append(m)
    return in_maps


def _run(inputs, trace=False):
    x_q = np.asarray(inputs["x_q"], dtype=np.float32)
    x_k = np.asarray(inputs["x_k"], dtype=np.float32)
    x_v = np.asarray(inputs["x_v"], dtype=np.float32)
    mask = np.asarray(inputs["mask"])
    wq = np.asarray(inputs["wq"], dtype=np.float32)
    bq = np.asarray(inputs["bq"], dtype=np.float32)
    wk = np.asarray(inputs["wk"], dtype=np.float32)
    bk = np.asarray(inputs["bk"], dtype=np.float32)
    wv = np.asarray(inputs["wv"], dtype=np.float32)
    bv = np.asarray(inputs["bv"], dtype=np.float32)
    wo = np.asarray(inputs["wo"], dtype=np.float32)
    bo = np.asarray(inputs["bo"], dtype=np.float32)

    nc = _get_nc()
    in_maps = _prep_in_maps(x_q, x_k, x_v, mask, wq, bq, wk, bk, wv, wo)
    kw = {}
    if trace:
        kw = dict(trace=True, trace_cores=[0], stitch_traces=False)
    res = run_bass_kernel_spmd(nc, in_maps, core_ids=list(range(8)), **kw)

    bo_eff = (bo + bv @ wo.T).astype(np.float32)
    out = np.empty((BS, SQ, DIM), np.float32)
    for b in range(BS):
        out[b] = res.results[2 * b]["out"] + res.results[2 * b + 1]["out"] + bo_eff
    return out, res


def kernel(**inputs):
    return _run(inputs, trace=False)[0]


# revision 21
# speedup vs baseline: 1.2861x; 1.1494x over previous
"""MultiHead scaled-dot-product attention on 8 Trainium2 NeuronCores.

Sharding: 8 cores = 4 batches x 2 head-halves. Core c handles batch c//2 and
heads [8*(c%2), 8*(c%2)+8) (512 of the 1024 feature columns of WQ/WK/WV,
and 512 rows of WO.T). Each core emits a partial output projection
(z_local @ wo.T_local, no bias); the host sums the two partials per batch and
adds bo_eff = bo + bv @ wo.T (the V-bias folds out of attention because
softmax rows sum to 1).

On-device layout per core (all matmul operands bf16, fp32 PSUM accumulate):
  Q^T, K^T: [512 feat, 2048 seq] (features on partitions, 4 head-pair tiles)
  V:        [2048 seq, 8 heads, 64+1] with a ones column -> PV matmul
            computes both z^T and the softmax denominator in one pass.
  S^T = K^T-chunks.T @ Q^T  (keys on partitions, queries free) so softmax
        needs no P transpose. Per key tile, the two heads of a pair run as
        row-packed (tile_position) matmuls into the two banks of one PSUM
        tile; one ScalarE exp covers both (scale=1/8 fused); the 0/1 bf16
        mask multiplies both via a step-0 broadcast AP.
  z^T feeds the WO matmul as the stationary operand directly. Softmax
  denominators are collected per query-tile and reciprocated in one batched
  DVE op to keep the slow reciprocal off the PE critical path (PE gaps
  > ~3.4us trip the HAM clock gate down to 1.2 GHz).
"""

import numpy as np
import ml_dtypes

import concourse.bass as bass
import concourse.mybir as mybir
from concourse import bacc
from concourse.bass_utils import run_bass_kernel_spmd
from concourse.tile import TileContext

BF16 = mybir.dt.bfloat16
F32 = mybir.dt.float32
NPBF16 = ml_dtypes.bfloat16

DIM = 1024
HEADS = 16
HD = 64
BS = 4
SQ = 2048
SK = 2048
LOCF = 512     # features per core (8 heads x 64)
NCH = DIM // 128   # 8 contraction chunks for projections
NQT = SQ // 512    # 4 query tiles
NKT = SK // 128    # 16 key tiles
NHP = 4            # head pairs per core

_BUILT = None


def _build():
    nc = bacc.Bacc("TRN2", target_bir_lowering=False, debug=False, num_devices=8)

    xqT = nc.dram_tensor("xqT", [DIM, SQ], BF16, kind="ExternalInput").ap()
    xkT = nc.dram_tensor("xkT", [DIM, SK], BF16, kind="ExternalInput").ap()
    xvT = nc.dram_tensor("xvT", [DIM, SK], BF16, kind="ExternalInput").ap()
    maskT = nc.dram_tensor("maskT", [SK, SQ], BF16, kind="ExternalInput").ap()
    wqT = nc.dram_tensor("wqT", [DIM, LOCF], BF16, kind="ExternalInput").ap()
    wkT = nc.dram_tensor("wkT", [DIM, LOCF], BF16, kind="ExternalInput").ap()
    wvT = nc.dram_tensor("wvT", [DIM, LOCF], BF16, kind="ExternalInput").ap()
    woT = nc.dram_tensor("woT", [LOCF, DIM], BF16, kind="ExternalInput").ap()
    bq_d = nc.dram_tensor("bq", [LOCF, 1], F32, kind="ExternalInput").ap()
    bk_d = nc.dram_tensor("bk", [LOCF, 1], F32, kind="ExternalInput").ap()
    # sel[h, hp, p] = 1 where head h's reciprocal should land on partition p
    # of head-pair hp's broadcast tile (p<64 -> even head, p>=64 -> odd head)
    sel_d = nc.dram_tensor("sel", [8, NHP, 128], F32, kind="ExternalInput").ap()
    out_d = nc.dram_tensor("out", [SQ, DIM], F32, kind="ExternalOutput").ap()

    Exp = mybir.ActivationFunctionType.Exp

    with TileContext(nc) as tc:
        from contextlib import ExitStack
        with ExitStack() as ctx:
            singles = ctx.enter_context(tc.tile_pool(name="singles", bufs=1))
            mask_pool = ctx.enter_context(tc.tile_pool(name="mask", bufs=3))
            pt_pool = ctx.enter_context(tc.tile_pool(name="pt", bufs=6))
            dn_pool = ctx.enter_context(tc.tile_pool(name="dn", bufs=2))
            osb_pool = ctx.enter_context(tc.tile_pool(name="osb", bufs=3))
            # PSUM: ps2 = 3 x 2-bank score tiles, psx = 2 x 1-bank tiles
            # shared by projections, PV accumulators, broadcast tiles and
            # WO outputs.
            ps2 = ctx.enter_context(tc.tile_pool(name="ps2", bufs=3, space="PSUM"))
            psx = ctx.enter_context(tc.tile_pool(name="psx", bufs=2, space="PSUM"))

            # --- weights / biases ---
            w_sb = {}
            for nm, dram in (("wq", wqT), ("wk", wkT), ("wv", wvT)):
                t = singles.tile([128, NCH, LOCF], BF16, tag=f"w_{nm}", name=f"w_{nm}")
                nc.sync.dma_start(out=t, in_=dram.rearrange("(c p) f -> p c f", p=128))
                w_sb[nm] = t
            woT_sb = singles.tile([128, 4, DIM], BF16, tag="wo")
            nc.sync.dma_start(out=woT_sb, in_=woT.rearrange("(c p) f -> p c f", p=128))
            bq_sb = singles.tile([128, 4, 1], F32, tag="bq")
            nc.sync.dma_start(out=bq_sb, in_=bq_d.rearrange("(m p) o -> p m o", p=128))
            bk_sb = singles.tile([128, 4, 1], F32, tag="bk")
            nc.sync.dma_start(out=bk_sb, in_=bk_d.rearrange("(m p) o -> p m o", p=128))
            sel_sb = singles.tile([8, NHP, 128], F32, tag="sel")
            nc.sync.dma_start(out=sel_sb, in_=sel_d)

            QT_sb = singles.tile([128, NHP, SQ], BF16, tag="QT")
            KT_sb = singles.tile([128, NHP, SK], BF16, tag="KT")
            ZT_sb = singles.tile([128, NHP, SQ], BF16, tag="ZT")
            V_sb = singles.tile([128, NKT, 8, HD + 1], BF16, tag="V")
            nc.vector.memset(V_sb[:, :, :, HD:HD + 1], 1.0)

            # --- phase A: projections, streamed in seq halves (1024 cols) ---
            HS = SQ // 2
            with tc.tile_pool(name="xc", bufs=8) as xc_pool:
                # K^T projection first (attention needs all keys first)
                for half in range(2):
                    h_sl = slice(half * HS, (half + 1) * HS)
                    xks = []
                    for c in range(NCH):
                        xt = xc_pool.tile([128, HS], BF16, tag="xc", name="xk")
                        nc.sync.dma_start(out=xt, in_=xkT[c * 128:(c + 1) * 128, h_sl])
                        xks.append(xt)
                    for m in range(NHP):
                        for nl in range(2):
                            n = half * 2 + nl
                            ps = psx.tile([128, 512], F32, tag="psx", name="psk")
                            for c in range(NCH):
                                nc.tensor.matmul(
                                    ps,
                                    lhsT=w_sb["wk"][:, c, m * 128:(m + 1) * 128],
                                    rhs=xks[c][:, nl * 512:(nl + 1) * 512],
                                    start=(c == 0),
                                    stop=(c == NCH - 1),
                                )
                            nc.vector.tensor_scalar_add(
                                out=KT_sb[:, m, n * 512:(n + 1) * 512],
                                in0=ps,
                                scalar1=bk_sb[:, m, :],
                            )

                # V projection: natural [seq, feat] layout (bias folded out)
                for half in range(2):
                    h_sl = slice(half * HS, (half + 1) * HS)
                    xvs = []
                    for c in range(NCH):
                        xt = xc_pool.tile([128, HS], BF16, tag="xc", name="xv")
                        nc.sync.dma_start(out=xt, in_=xvT[c * 128:(c + 1) * 128, h_sl])
                        xvs.append(xt)
                    for ml in range(NKT // 2):
                        m = half * (NKT // 2) + ml
                        ps = psx.tile([128, 512], F32, tag="psx", name="psv")
                        for c in range(NCH):
                            nc.tensor.matmul(
                                ps,
                                lhsT=xvs[c][:, ml * 128:(ml + 1) * 128],
                                rhs=w_sb["wv"][:, c, :],
                                start=(c == 0),
                                stop=(c == NCH - 1),
                            )
                        nc.vector.tensor_copy(
                            out=V_sb[:, m, :, 0:HD],
                            in_=ps.rearrange("p (h d) -> p h d", h=8),
                        )

                # Q^T projection, seq-tile-major so qt=0 attention can start
                for half in range(2):
                    h_sl = slice(half * HS, (half + 1) * HS)
                    xqs = []
                    for c in range(NCH):
                        xt = xc_pool.tile([128, HS], BF16, tag="xc", name="xq")
                        nc.sync.dma_start(out=xt, in_=xqT[c * 128:(c + 1) * 128, h_sl])
                        xqs.append(xt)
                    for nl in range(2):
                        n = half * 2 + nl
                        for m in range(NHP):
                            ps = psx.tile([128, 512], F32, tag="psx", name="psq")
                            for c in range(NCH):
                                nc.tensor.matmul(
                                    ps,
                                    lhsT=w_sb["wq"][:, c, m * 128:(m + 1) * 128],
                                    rhs=xqs[c][:, nl * 512:(nl + 1) * 512],
                                    start=(c == 0),
                                    stop=(c == NCH - 1),
                                )
                            nc.vector.tensor_scalar_add(
                                out=QT_sb[:, m, n * 512:(n + 1) * 512],
                                in0=ps,
                                scalar1=bq_sb[:, m, :],
                            )

            # --- attention + output projection, per query tile.
            # norm+WO of tile qt-1 is emitted inside qt's head loop so the
            # PE never drains while the normalization chain resolves. ---
            def emit_norm_wo(qt, rall):
                q_sl = slice(qt * 512, (qt + 1) * 512)
                for hp in range(NHP):
                    # per-pair reciprocal broadcast via selector matmul:
                    # rbp[p, q] = rall[2*hp + (p >= 64), q]  (lands in PSUM)
                    rbp = psx.tile([128, 512], F32, tag="psx", name="rbp")
                    nc.tensor.matmul(
                        rbp, lhsT=sel_sb[:, hp, :], rhs=rall,
                        start=True, stop=True,
                    )
                    nc.vector.tensor_mul(
                        out=ZT_sb[:, hp, q_sl],
                        in0=ZT_sb[:, hp, q_sl],
                        in1=rbp,
                    )
                for st in range(4):
                    s0 = qt * 512 + st * 128
                    for n in range(2):
                        po = psx.tile([128, 512], F32, tag="psx", name="po")
                        for c4 in range(NHP):
                            nc.tensor.matmul(
                                po,
                                lhsT=ZT_sb[:, c4, s0:s0 + 128],
                                rhs=woT_sb[:, c4, n * 512:(n + 1) * 512],
                                start=(c4 == 0),
                                stop=(c4 == NHP - 1),
                            )
                        ot = osb_pool.tile([128, 512], F32, tag="osb", name="ot")
                        nc.vector.tensor_copy(out=ot, in_=po)
                        nc.sync.dma_start(
                            out=out_d[s0:s0 + 128, n * 512:(n + 1) * 512], in_=ot
                        )

            pending = None  # (qt, rall) awaiting norm + WO
            for qt in range(NQT):
                q_sl = slice(qt * 512, (qt + 1) * 512)
                mks = []
                for kh in range(2):
                    mk = mask_pool.tile([128, NKT // 2, 512], BF16, tag="mask",
                                        name="mk")
                    nc.sync.dma_start(
                        out=mk,
                        in_=maskT.rearrange("(kt p) q -> p kt q", p=128)[
                            :, kh * (NKT // 2):(kh + 1) * (NKT // 2), q_sl],
                    )
                    mks.append(mk)
                dn = dn_pool.tile([8, 512], F32, tag="dn", name="dn")
                for hp in range(NHP):
                    pz = [
                        psx.tile([HD + 1, 512], F32, tag="psx", name="pz0"),
                        psx.tile([HD + 1, 512], F32, tag="psx", name="pz1"),
                    ]
                    for kt in range(NKT):
                        ss = ps2.tile([128, 2, 512], F32, tag="ps2", name="ss")
                        for hl in range(2):
                            p_sl = slice(hl * 64, (hl + 1) * 64)
                            nc.tensor.matmul(
                                ss[:, hl, :],
                                lhsT=KT_sb[p_sl, hp, kt * 128:(kt + 1) * 128],
                                rhs=QT_sb[p_sl, hp, q_sl],
                                start=True,
                                stop=True,
                            )
                        pt = pt_pool.tile([128, 2, 512], BF16, tag="pt", name="pt")
                        nc.scalar.activation(out=pt, in_=ss, func=Exp, scale=0.125)
                        mkh = mks[kt // (NKT // 2)]
                        ktl = kt % (NKT // 2)
                        nc.vector.tensor_mul(
                            out=pt,
                            in0=pt,
                            in1=mkh[:, ktl:ktl + 1, :].to_broadcast((128, 2, 512)),
                        )
                        for hl in range(2):
                            nc.tensor.matmul(
                                pz[hl],
                                lhsT=V_sb[:, kt, hp * 2 + hl, :],
                                rhs=pt[:, hl, :],
                                start=(kt == 0),
                                stop=(kt == NKT - 1),
                            )
                    for hl in range(2):
                        h = hp * 2 + hl
                        p_sl = slice(hl * 64, (hl + 1) * 64)
                        # stash unnormalized z^T; free the PSUM bank fast
                        nc.vector.tensor_copy(
                            out=ZT_sb[p_sl, hp, q_sl], in_=pz[hl][0:HD, :]
                        )
                        dnst = dn_pool.tile([1, 512], F32, tag="dnst", name="dnst")
                        nc.vector.tensor_copy(out=dnst, in_=pz[hl][HD:HD + 1, :])
                        nc.sync.dma_start(out=dn[h:h + 1, :], in_=dnst)
                    if hp == 0 and pending is not None:
                        emit_norm_wo(*pending)
                        pending = None
                # one batched reciprocal for all 8 heads of this q tile
                rall = dn_pool.tile([8, 512], F32, tag="rall", name="rall")
                nc.vector.reciprocal(out=rall, in_=dn)
                pending = (qt, rall)
            emit_norm_wo(*pending)

    nc.compile()
    return nc


def _get_nc():
    global _BUILT
    if _BUILT is None:
        _BUILT = _build()
    return _BUILT


def _prep_in_maps(x_q, x_k, x_v, mask, wq, bq, wk, bk, wv, wo):
    per_batch = []
    for b in range(BS):
        per_batch.append({
            "xqT": np.ascontiguousarray(x_q[b].T).astype(NPBF16),
            "xkT": np.ascontiguousarray(x_k[b].T).astype(NPBF16),
            "xvT": np.ascontiguousarray(x_v[b].T).astype(NPBF16),
            "maskT": np.ascontiguousarray((mask[b] != 0).T).astype(NPBF16),
        })
    sel = np.zeros((8, NHP, 128), np.float32)
    for hp in range(NHP):
        sel[2 * hp, hp, 0:HD] = 1.0
        sel[2 * hp + 1, hp, HD:128] = 1.0
    per_half = []
    for hh in range(2):
        sl = slice(hh * LOCF, (hh + 1) * LOCF)
        per_half.append({
            "wqT": np.ascontiguousarray(wq.T[:, sl]).astype(NPBF16),
            "wkT": np.ascontiguousarray(wk.T[:, sl]).astype(NPBF16),
            "wvT": np.ascontiguousarray(wv.T[:, sl]).astype(NPBF16),
            "woT": np.ascontiguousarray(wo.T[sl, :]).astype(NPBF16),
            "bq": np.ascontiguousarray(bq[sl]).reshape(LOCF, 1).astype(np.float32),
            "bk": np.ascontiguousarray(bk[sl]).reshape(LOCF, 1).astype(np.float32),
        })
    in_maps = []
    for c in range(8):
        m = dict(per_batch[c // 2])
        m.update(per_half[c % 2])
        m["sel"] = sel
        in_maps.append(m)
    return in_maps


def _run(inputs, trace=False):
    x_q = np.asarray(inputs["x_q"], dtype=np.float32)
    x_k = np.asarray(inputs["x_k"], dtype=np.float32)
    x_v = np.asarray(inputs["x_v"], dtype=np.float32)
    mask = np.asarray(inputs["mask"])
    wq = np.asarray(inputs["wq"], dtype=np.float32)
    bq = np.asarray(inputs["bq"], dtype=np.float32)
    wk = np.asarray(inputs["wk"], dtype=np.float32)
    bk = np.asarray(inputs["bk"], dtype=np.float32)
    wv = np.asarray(inputs["wv"], dtype=np.float32)
    bv = np.asarray(inputs["bv"], dtype=np.float32)
    wo = np.asarray(inputs["wo"], dtype=np.float32)
    bo = np.asarray(inputs["bo"], dtype=np.float32)

    nc = _get_nc()
    in_maps = _prep_in_maps(x_q, x_k, x_v, mask, wq, bq, wk, bk, wv, wo)
    kw = {}
    if trace:
        kw = dict(trace=True, trace_cores=[0], stitch_traces=False)
    res = run_bass_kernel_spmd(nc, in_maps, core_ids=list(range(8)), **kw)

    bo_eff = (bo + bv @ wo.T).astype(np.float32)
    out = np.empty((BS, SQ, DIM), np.float32)
    for b in range(BS):
        out[b] = res.results[2 * b]["out"] + res.results[2 * b + 1]["out"] + bo_eff
    return out, res


def kernel(**inputs):
    return _run(inputs, trace=False)[0]


# revision 24
# speedup vs baseline: 1.3630x; 1.0597x over previous
"""MultiHead scaled-dot-product attention on 8 Trainium2 NeuronCores.

Sharding: 8 cores = 4 batches x 2 head-halves. Core c handles batch c//2 and
heads [8*(c%2), 8*(c%2)+8) (512 of the 1024 feature columns of WQ/WK/WV,
and 512 rows of WO.T). Each core emits a partial output projection
(z_local @ wo.T_local, no bias); the host sums the two partials per batch and
adds bo_eff = bo + bv @ wo.T (the V-bias folds out of attention because
softmax rows sum to 1).

On-device layout per core (all matmul operands bf16, fp32 PSUM accumulate):
  Q^T, K^T: [512 feat, 2048 seq] (features on partitions, 4 head-pair tiles)
  V:        [2048 seq, 8 heads, 64+1] with a ones column -> PV matmul
            computes both z^T and the softmax denominator in one pass.
  S^T = K^T-chunks.T @ Q^T  (keys on partitions, queries free) so softmax
        needs no P transpose. Per key tile, the two heads of a pair run as
        row-packed (tile_position) matmuls into the two banks of one PSUM
        tile; one ScalarE exp covers both (scale=1/8 fused); the 0/1 bf16
        mask multiplies both via a step-0 broadcast AP.
  z^T feeds the WO matmul as the stationary operand directly. Softmax
  denominators are collected per query-tile and reciprocated in one batched
  DVE op to keep the slow reciprocal off the PE critical path (PE gaps
  > ~3.4us trip the HAM clock gate down to 1.2 GHz).
"""

import numpy as np
import ml_dtypes

import concourse.bass as bass
import concourse.mybir as mybir
from concourse import bacc
from concourse.bass_utils import run_bass_kernel_spmd
from concourse.tile import TileContext

BF16 = mybir.dt.bfloat16
F32 = mybir.dt.float32
NPBF16 = ml_dtypes.bfloat16

DIM = 1024
HEADS = 16
HD = 64
BS = 4
SQ = 2048
SK = 2048
LOCF = 512     # features per core (8 heads x 64)
NCH = DIM // 128   # 8 contraction chunks for projections
NQT = SQ // 512    # 4 query tiles
NKT = SK // 128    # 16 key tiles
NHP = 4            # head pairs per core

_BUILT = None


def _build():
    nc = bacc.Bacc("TRN2", target_bir_lowering=False, debug=False, num_devices=8)

    xqT = nc.dram_tensor("xqT", [DIM, SQ], BF16, kind="ExternalInput").ap()
    xkT = nc.dram_tensor("xkT", [DIM, SK], BF16, kind="ExternalInput").ap()
    xvT = nc.dram_tensor("xvT", [DIM, SK], BF16, kind="ExternalInput").ap()
    maskT = nc.dram_tensor("maskT", [SK, SQ], BF16, kind="ExternalInput").ap()
    wqT = nc.dram_tensor("wqT", [DIM, LOCF], BF16, kind="ExternalInput").ap()
    wkT = nc.dram_tensor("wkT", [DIM, LOCF], BF16, kind="ExternalInput").ap()
    wvT = nc.dram_tensor("wvT", [DIM, LOCF], BF16, kind="ExternalInput").ap()
    woT = nc.dram_tensor("woT", [LOCF, DIM], BF16, kind="ExternalInput").ap()
    bq_d = nc.dram_tensor("bq", [LOCF, 1], F32, kind="ExternalInput").ap()
    bk_d = nc.dram_tensor("bk", [LOCF, 1], F32, kind="ExternalInput").ap()
    # sel[h, hp, p] = 1 where head h's reciprocal should land on partition p
    # of head-pair hp's broadcast tile (p<64 -> even head, p>=64 -> odd head)
    sel_d = nc.dram_tensor("sel", [8, NHP, 128], F32, kind="ExternalInput").ap()
    out_d = nc.dram_tensor("out", [SQ, DIM], F32, kind="ExternalOutput").ap()

    Exp = mybir.ActivationFunctionType.Exp

    with TileContext(nc) as tc:
        from contextlib import ExitStack
        with ExitStack() as ctx:
            singles = ctx.enter_context(tc.tile_pool(name="singles", bufs=1))
            mask_pool = ctx.enter_context(tc.tile_pool(name="mask", bufs=3))
            pt_pool = ctx.enter_context(tc.tile_pool(name="pt", bufs=6))
            dn_pool = ctx.enter_context(tc.tile_pool(name="dn", bufs=2))
            osb_pool = ctx.enter_context(tc.tile_pool(name="osb", bufs=3))
            # PSUM: ps2 = 3 x 2-bank score tiles, psx = 2 x 1-bank tiles
            # shared by projections, PV accumulators, broadcast tiles and
            # WO outputs.
            ps2 = ctx.enter_context(tc.tile_pool(name="ps2", bufs=3, space="PSUM"))
            psx = ctx.enter_context(tc.tile_pool(name="psx", bufs=2, space="PSUM"))

            # --- weights / biases (spread across the two HWDGE queues) ---
            w_sb = {}
            for nm, dram, eng in (
                ("wk", wkT, nc.sync),
                ("wq", wqT, nc.scalar),
                ("wv", wvT, nc.scalar),
            ):
                t = singles.tile([128, NCH, LOCF], BF16, tag=f"w_{nm}", name=f"w_{nm}")
                eng.dma_start(out=t, in_=dram.rearrange("(c p) f -> p c f", p=128))
                w_sb[nm] = t
            woT_sb = singles.tile([128, 4, DIM], BF16, tag="wo")
            nc.scalar.dma_start(out=woT_sb, in_=woT.rearrange("(c p) f -> p c f", p=128))
            bq_sb = singles.tile([128, 4, 1], F32, tag="bq")
            nc.scalar.dma_start(out=bq_sb, in_=bq_d.rearrange("(m p) o -> p m o", p=128))
            bk_sb = singles.tile([128, 4, 1], F32, tag="bk")
            nc.sync.dma_start(out=bk_sb, in_=bk_d.rearrange("(m p) o -> p m o", p=128))
            sel_sb = singles.tile([8, NHP, 128], F32, tag="sel")
            nc.sync.dma_start(out=sel_sb, in_=sel_d)

            QT_sb = singles.tile([128, NHP, SQ], BF16, tag="QT")
            KT_sb = singles.tile([128, NHP, SK], BF16, tag="KT")
            ZT_sb = singles.tile([128, NHP, SQ], BF16, tag="ZT")
            V_sb = singles.tile([128, NKT, 8, HD + 1], BF16, tag="V")
            nc.vector.memset(V_sb[:, :, :, HD:HD + 1], 1.0)

            # --- phase A: projections, streamed in seq halves (1024 cols).
            # Order: K (both halves) -> Q (first half: qt 0,1) -> V; the
            # second Q half is deferred into the attention stream. ---
            HS = SQ // 2
            xc_pool = ctx.enter_context(tc.tile_pool(name="xc", bufs=8))
            xq_pool = ctx.enter_context(tc.tile_pool(name="xq", bufs=8))

            def emit_k_proj(half):
                h_sl = slice(half * HS, (half + 1) * HS)
                xks = []
                for c in range(NCH):
                    xt = xc_pool.tile([128, HS], BF16, tag="xc", name="xk")
                    nc.sync.dma_start(out=xt, in_=xkT[c * 128:(c + 1) * 128, h_sl])
                    xks.append(xt)
                for m in range(NHP):
                    for nl in range(2):
                        n = half * 2 + nl
                        ps = psx.tile([128, 512], F32, tag="psx", name="psk")
                        for c in range(NCH):
                            nc.tensor.matmul(
                                ps,
                                lhsT=w_sb["wk"][:, c, m * 128:(m + 1) * 128],
                                rhs=xks[c][:, nl * 512:(nl + 1) * 512],
                                start=(c == 0),
                                stop=(c == NCH - 1),
                            )
                        nc.vector.tensor_scalar_add(
                            out=KT_sb[:, m, n * 512:(n + 1) * 512],
                            in0=ps,
                            scalar1=bk_sb[:, m, :],
                        )

            def emit_q_proj(half):
                h_sl = slice(half * HS, (half + 1) * HS)
                xqs = []
                for c in range(NCH):
                    xt = xq_pool.tile([128, HS], BF16, tag="xq", name="xq")
                    nc.scalar.dma_start(out=xt, in_=xqT[c * 128:(c + 1) * 128, h_sl])
                    xqs.append(xt)
                for nl in range(2):
                    n = half * 2 + nl
                    for m in range(NHP):
                        ps = psx.tile([128, 512], F32, tag="psx", name="psq")
                        for c in range(NCH):
                            nc.tensor.matmul(
                                ps,
                                lhsT=w_sb["wq"][:, c, m * 128:(m + 1) * 128],
                                rhs=xqs[c][:, nl * 512:(nl + 1) * 512],
                                start=(c == 0),
                                stop=(c == NCH - 1),
                            )
                        nc.vector.tensor_scalar_add(
                            out=QT_sb[:, m, n * 512:(n + 1) * 512],
                            in0=ps,
                            scalar1=bq_sb[:, m, :],
                        )

            def emit_v_proj(half):
                h_sl = slice(half * HS, (half + 1) * HS)
                xvs = []
                for c in range(NCH):
                    xt = xc_pool.tile([128, HS], BF16, tag="xc", name="xv")
                    nc.scalar.dma_start(out=xt, in_=xvT[c * 128:(c + 1) * 128, h_sl])
                    xvs.append(xt)
                for ml in range(NKT // 2):
                    m = half * (NKT // 2) + ml
                    ps = psx.tile([128, 512], F32, tag="psx", name="psv")
                    for c in range(NCH):
                        nc.tensor.matmul(
                            ps,
                            lhsT=xvs[c][:, ml * 128:(ml + 1) * 128],
                            rhs=w_sb["wv"][:, c, :],
                            start=(c == 0),
                            stop=(c == NCH - 1),
                        )
                    nc.scalar.copy(
                        out=V_sb[:, m, :, 0:HD],
                        in_=ps.rearrange("p (h d) -> p h d", h=8),
                    )

            emit_k_proj(0)
            emit_k_proj(1)
            emit_q_proj(0)
            emit_v_proj(0)
            emit_v_proj(1)

            # --- attention + output projection, per query tile.
            # norm+WO of tile qt-1 is emitted inside qt's head loop so the
            # PE never drains while the normalization chain resolves. ---
            def emit_norm_wo(qt, rall):
                q_sl = slice(qt * 512, (qt + 1) * 512)
                for hp in range(NHP):
                    # per-pair reciprocal broadcast via selector matmul:
                    # rbp[p, q] = rall[2*hp + (p >= 64), q]  (lands in PSUM)
                    rbp = psx.tile([128, 512], F32, tag="psx", name="rbp")
                    nc.tensor.matmul(
                        rbp, lhsT=sel_sb[:, hp, :], rhs=rall,
                        start=True, stop=True,
                    )
                    nc.vector.tensor_mul(
                        out=ZT_sb[:, hp, q_sl],
                        in0=ZT_sb[:, hp, q_sl],
                        in1=rbp,
                    )
                for st in range(4):
                    s0 = qt * 512 + st * 128
                    for n in range(2):
                        po = psx.tile([128, 512], F32, tag="psx", name="po")
                        for c4 in range(NHP):
                            nc.tensor.matmul(
                                po,
                                lhsT=ZT_sb[:, c4, s0:s0 + 128],
                                rhs=woT_sb[:, c4, n * 512:(n + 1) * 512],
                                start=(c4 == 0),
                                stop=(c4 == NHP - 1),
                            )
                        ot = osb_pool.tile([128, 512], F32, tag="osb", name="ot")
                        nc.vector.tensor_copy(out=ot, in_=po)
                        nc.sync.dma_start(
                            out=out_d[s0:s0 + 128, n * 512:(n + 1) * 512], in_=ot
                        )

            pending = None  # (qt, rall) awaiting norm + WO
            for qt in range(NQT):
                q_sl = slice(qt * 512, (qt + 1) * 512)
                mks = []
                for kh in range(2):
                    mk = mask_pool.tile([128, NKT // 2, 512], BF16, tag="mask",
                                        name="mk")
                    nc.sync.dma_start(
                        out=mk,
                        in_=maskT.rearrange("(kt p) q -> p kt q", p=128)[
                            :, kh * (NKT // 2):(kh + 1) * (NKT // 2), q_sl],
                    )
                    mks.append(mk)
                dn = dn_pool.tile([8, 512], F32, tag="dn", name="dn")
                for hp in range(NHP):
                    pz = [
                        psx.tile([HD + 1, 512], F32, tag="psx", name="pz0"),
                        psx.tile([HD + 1, 512], F32, tag="psx", name="pz1"),
                    ]
                    for kt in range(NKT):
                        ss = ps2.tile([128, 2, 512], F32, tag="ps2", name="ss")
                        for hl in range(2):
                            p_sl = slice(hl * 64, (hl + 1) * 64)
                            nc.tensor.matmul(
                                ss[:, hl, :],
                                lhsT=KT_sb[p_sl, hp, kt * 128:(kt + 1) * 128],
                                rhs=QT_sb[p_sl, hp, q_sl],
                                start=True,
                                stop=True,
                            )
                        pt = pt_pool.tile([128, 2, 512], BF16, tag="pt", name="pt")
                        nc.scalar.activation(out=pt, in_=ss, func=Exp, scale=0.125)
                        mkh = mks[kt // (NKT // 2)]
                        ktl = kt % (NKT // 2)
                        nc.vector.tensor_mul(
                            out=pt,
                            in0=pt,
                            in1=mkh[:, ktl:ktl + 1, :].to_broadcast((128, 2, 512)),
                        )
                        for hl in range(2):
                            nc.tensor.matmul(
                                pz[hl],
                                lhsT=V_sb[:, kt, hp * 2 + hl, :],
                                rhs=pt[:, hl, :],
                                start=(kt == 0),
                                stop=(kt == NKT - 1),
                            )
                    for hl in range(2):
                        h = hp * 2 + hl
                        p_sl = slice(hl * 64, (hl + 1) * 64)
                        # stash unnormalized z^T; free the PSUM bank fast
                        nc.vector.tensor_copy(
                            out=ZT_sb[p_sl, hp, q_sl], in_=pz[hl][0:HD, :]
                        )
                        dnst = dn_pool.tile([1, 512], F32, tag="dnst", name="dnst")
                        nc.vector.tensor_copy(out=dnst, in_=pz[hl][HD:HD + 1, :])
                        nc.sync.dma_start(out=dn[h:h + 1, :], in_=dnst)
                    if hp == 0 and pending is not None:
                        emit_norm_wo(*pending)
                        pending = None
                    if hp == 1 and qt == 1:
                        # deferred second-half Q projection (for qt 2, 3)
                        emit_q_proj(1)
                # one batched reciprocal for all 8 heads of this q tile
                rall = dn_pool.tile([8, 512], F32, tag="rall", name="rall")
                nc.vector.reciprocal(out=rall, in_=dn)
                pending = (qt, rall)
            emit_norm_wo(*pending)

    nc.compile()
    return nc


def _get_nc():
    global _BUILT
    if _BUILT is None:
        _BUILT = _build()
    return _BUILT


def _prep_in_maps(x_q, x_k, x_v, mask, wq, bq, wk, bk, wv, wo):
    per_batch = []
    for b in range(BS):
        per_batch.append({
            "xqT": np.ascontiguousarray(x_q[b].T).astype(NPBF16),
            "xkT": np.ascontiguousarray(x_k[b].T).astype(NPBF16),
            "xvT": np.ascontiguousarray(x_v[b].T).astype(NPBF16),
            "maskT": np.ascontiguousarray((mask[b] != 0).T).astype(NPBF16),
        })
    sel = np.zeros((8, NHP, 128), np.float32)
    for hp in range(NHP):
        sel[2 * hp, hp, 0:HD] = 1.0
        sel[2 * hp + 1, hp, HD:128] = 1.0
    per_half = []
    for hh in range(2):
        sl = slice(hh * LOCF, (hh + 1) * LOCF)
        per_half.append({
            "wqT": np.ascontiguousarray(wq.T[:, sl]).astype(NPBF16),
            "wkT": np.ascontiguousarray(wk.T[:, sl]).astype(NPBF16),
            "wvT": np.ascontiguousarray(wv.T[:, sl]).astype(NPBF16),
            "woT": np.ascontiguousarray(wo.T[sl, :]).astype(NPBF16),
            "bq": np.ascontiguousarray(bq[sl]).reshape(LOCF, 1).astype(np.float32),
            "bk": np.ascontiguousarray(bk[sl]).reshape(LOCF, 1).astype(np.float32),
        })
    in_maps = []
    for c in range(8):
        m = dict(per_batch[c // 2])
        m.update(per_half[c % 2])
        m["sel"] = sel
        in_maps.append(m)
    return in_maps


def _run(inputs, trace=False):
    x_q = np.asarray(inputs["x_q"], dtype=np.float32)
    x_k = np.asarray(inputs["x_k"], dtype=np.float32)
    x_v = np.asarray(inputs["x_v"], dtype=np.float32)
    mask = np.asarray(inputs["mask"])
    wq = np.asarray(inputs["wq"], dtype=np.float32)
    bq = np.asarray(inputs["bq"], dtype=np.float32)
    wk = np.asarray(inputs["wk"], dtype=np.float32)
    bk = np.asarray(inputs["bk"], dtype=np.float32)
    wv = np.asarray(inputs["wv"], dtype=np.float32)
    bv = np.asarray(inputs["bv"], dtype=np.float32)
    wo = np.asarray(inputs["wo"], dtype=np.float32)
    bo = np.asarray(inputs["bo"], dtype=np.float32)

    nc = _get_nc()
    in_maps = _prep_in_maps(x_q, x_k, x_v, mask, wq, bq, wk, bk, wv, wo)
    kw = {}
    if trace:
        kw = dict(trace=True, trace_cores=[0], stitch_traces=False)
    res = run_bass_kernel_spmd(nc, in_maps, core_ids=list(range(8)), **kw)

    bo_eff = (bo + bv @ wo.T).astype(np.float32)
    out = np.empty((BS, SQ, DIM), np.float32)
    for b in range(BS):
        out[b] = res.results[2 * b]["out"] + res.results[2 * b + 1]["out"] + bo_eff
    return out, res


def kernel(**inputs):
    return _run(inputs, trace=False)[0]
